# revision 1
# baseline (speedup 1.0000x reference)
"""Sliding-window causal self-attention for Trainium2, 8 NeuronCores.

Problem: B=4, T=2048, C=1024, 16 heads x 64 dim, window=256 causal band.
  qkv = x @ W_qkv.T ; windowed-causal attention ; out = y @ W_proj.T

Sharding: 8 cores = 4 batches x 2 sequence halves (1024 queries each).
Each core receives a 1280-row "extended" slice of its batch's x (256
preceding rows for the attention window, zero-padded for the first half)
and computes its 1024 output rows with zero cross-core communication.

On-core layout is fully transposed (channel-major):
  - host passes x_ext^T [C, 1280], W_qkv^T [C, 3C], W_proj^T [C, C]
  - q^T,k^T computed head-dim-major: [c, t];  V in natural [t, c] layout,
    augmented with a ones column per head (65 cols/head) so the AV matmul
    also produces the softmax denominator (row 64 of each PSUM tile).
  - scores computed as S^T[key, query] per 256-query chunk over its
    512-key window (4 key subtiles of 128); exp on ScalarE (scale=1/8
    folded in); band masks are 0/1 input tensors applied multiplicatively
    post-exp on VectorE; AV accumulates over the 4 subtiles.
  - y^T/rowsum -> reciprocal -> normalize -> proj -> outT [C, 1024];
    host transposes back.

All matmuls use float32r (TF32-like, ~1e-4 rel err, full PE speed at
moving-dim >= 256).
"""

import numpy as np
from contextlib import ExitStack

import concourse.bass as bass
import concourse.tile as tile
import concourse.mybir as mybir
from concourse import bacc
from concourse.tile import add_dep_helper
from concourse import bass_utils

F32 = mybir.dt.float32
F32R = mybir.dt.float32r
AF = mybir.ActivationFunctionType

C = 1024
HEADS = 16
D = 64
WINDOW = 256
QC = 256            # queries per chunk
PAD = WINDOW        # leading ext rows
CO = C // 128       # 8

FULL_MASK = False   # debug: disable half-width exp/mask scheme
PAIR_S = False       # debug: share one PSUM bank between two heads' scores


def _build_body(tc, xT, wqkvT, wprojT, masks, outT, TQ):
    nc = tc.nc
    EXT = TQ + PAD
    EO = EXT // 128
    NCH = TQ // QC

    with ExitStack() as outer:
        kt_pool = outer.enter_context(tc.tile_pool(name="kt", bufs=1))
        qt_pool = outer.enter_context(tc.tile_pool(name="qt", bufs=1))
        v_pool = outer.enter_context(tc.tile_pool(name="vv", bufs=1))
        const_pool = outer.enter_context(tc.tile_pool(name="const", bufs=1))

        kT = kt_pool.tile([128, CO, EXT], F32R)
        qT = qt_pool.tile([128, CO, TQ], F32R)
        V = v_pool.tile([128, EO, HEADS * (D + 1)], F32R)
        mask_sb = const_pool.tile([128, 8, QC], F32)
        ones_col = const_pool.tile([128, 1], F32)
        zeros_qc = const_pool.tile([128, QC], F32R)

        nc.vector.memset(ones_col[:], 1.0)
        zf = const_pool.tile([128, 1], F32)
        nc.vector.memset(zf[:], 0.0)
        nc.vector.tensor_copy(zeros_qc[:], zf[:, 0:1].broadcast_to([128, QC]))

        # ones columns of V_aug (col D of each head's 65-col group)
        v_ones_view = V[:].rearrange("p e (h x) -> p e h x", x=D + 1)[:, :, :, D]
        nc.vector.tensor_copy(
            v_ones_view, ones_col[:, 0:1].broadcast_to([128, EO, HEADS])
        )

        # ---------------- Phase A: qkv projections ----------------
        with ExitStack() as ctx:
            x_pool = ctx.enter_context(tc.tile_pool(name="xx", bufs=1))
            w_pool = ctx.enter_context(tc.tile_pool(name="wA", bufs=3))
            psA = ctx.enter_context(tc.tile_pool(name="psA", bufs=4, space="PSUM"))

            # slice the x load; only the first 512 cols gate the first
            # matmuls, the rest queues behind the first t-chunk's W blocks
            xt = x_pool.tile([128, CO, EXT], F32R)
            xTr = xT.rearrange("(o p) t -> p o t", p=128)
            nc.sync.dma_start(xt[:, :, 0:512], xTr[:, :, 0:512])

            # kT: head-dim-major keys; qT likewise for queries only.
            # t-chunk-outer so the first matmuls only wait on the first two
            # x slices (W blocks are re-streamed per t-chunk; DMA has slack).
            for t0 in range(0, EXT, 512):
                tn = min(512, EXT - t0)
                for co in range(CO):
                    wk = w_pool.tile([128, CO, 128], F32R, tag="wblk")
                    nc.sync.dma_start(
                        wk[:],
                        wqkvT[:, C + co * 128 : C + (co + 1) * 128].rearrange(
                            "(o p) c -> p o c", p=128
                        ),
                    )
                    if t0 == 0 and co == CO - 1 and EXT > 512:
                        # remaining x slices, queued behind t-chunk 0's W blocks
                        nc.sync.dma_start(xt[:, :, 512:EXT], xTr[:, :, 512:EXT])
                    ps = psA.tile([128, 512], F32, tag="ps")
                    for ki in range(CO):
                        nc.tensor.matmul(
                            ps[:, :tn],
                            wk[:, ki, :],
                            xt[:, ki, t0 : t0 + tn],
                            start=(ki == 0),
                            stop=(ki == CO - 1),
                        )
                    nc.scalar.activation(kT[:, co, t0 : t0 + tn], ps[:, :tn], AF.Copy)

            for t0 in range(0, TQ, 512):
                tn = min(512, TQ - t0)
                for co in range(CO):
                    wq = w_pool.tile([128, CO, 128], F32R, tag="wblk")
                    nc.sync.dma_start(
                        wq[:],
                        wqkvT[:, co * 128 : (co + 1) * 128].rearrange(
                            "(o p) c -> p o c", p=128
                        ),
                    )
                    ps = psA.tile([128, 512], F32, tag="ps")
                    for ki in range(CO):
                        nc.tensor.matmul(
                            ps[:, :tn],
                            wq[:, ki, :],
                            xt[:, ki, PAD + t0 : PAD + t0 + tn],
                            start=(ki == 0),
                            stop=(ki == CO - 1),
                        )
                    nc.scalar.activation(qT[:, co, t0 : t0 + tn], ps[:, :tn], AF.Copy)

            # V natural layout [t, c], 256-col chunks (4 heads each)
            for cb in range(4):
                wv = w_pool.tile([128, CO, 256], F32R, tag="wvblk")
                nc.sync.dma_start(
                    wv[:],
                    wqkvT[:, 2 * C + cb * 256 : 2 * C + (cb + 1) * 256].rearrange(
                        "(o p) c -> p o c", p=128
                    ),
                )
                for eo in range(EO):
                    ps = psA.tile([128, 256], F32, tag="psv")
                    for ki in range(CO):
                        nc.tensor.matmul(
                            ps[:],
                            xt[:, ki, eo * 128 : (eo + 1) * 128],
                            wv[:, ki, :],
                            start=(ki == 0),
                            stop=(ki == CO - 1),
                        )
                    v_dst = V[:].rearrange("p e (h x) -> p e h x", x=D + 1)[
                        :, eo, 4 * cb : 4 * cb + 4, 0:D
                    ]
                    nc.scalar.activation(
                        v_dst, ps[:].rearrange("p (h d) -> p h d", d=D), AF.Copy
                    )

        # ---------------- Phase B: attention + projection ----------------
        with ExitStack() as ctx:
            wp_pool = ctx.enter_context(tc.tile_pool(name="wp", bufs=1))
            pm_pool = ctx.enter_context(tc.tile_pool(name="pm", bufs=10))
            ostage_pool = ctx.enter_context(tc.tile_pool(name="ost", bufs=2))
            yu_pool = ctx.enter_context(tc.tile_pool(name="yu", bufs=4))
            ysb_pool = ctx.enter_context(tc.tile_pool(name="ysb", bufs=2))
            r_pool = ctx.enter_context(tc.tile_pool(name="rr", bufs=4))
            psS = ctx.enter_context(tc.tile_pool(name="psS", bufs=2, space="PSUM"))
            psY = ctx.enter_context(tc.tile_pool(name="psY", bufs=3, space="PSUM"))
            psP = ctx.enter_context(tc.tile_pool(name="psP", bufs=1, space="PSUM"))

            nc.sync.dma_start(mask_sb[:], masks.rearrange("m s p q -> p (m s) q"))
            wp_sb = wp_pool.tile([128, CO, C], F32R)
            nc.sync.dma_start(wp_sb[:], wprojT.rearrange("(o p) c -> p o c", p=128))

            for ch in range(NCH):
                mset = 0 if ch == 0 else 4  # mask set index base
                y_sb = ysb_pool.tile([128, CO, QC], F32R)

                for hp in range(8):
                    h0, h1 = 2 * hp, 2 * hp + 1
                    pms = {}
                    for s in range(4):
                        # two heads share one PSUM bank: head h1's matmul uses
                        # start=False (+skip_group_check) so it overwrites its
                        # untouched half without re-arming the bank's
                        # pending-zero region.
                        if PAIR_S:
                            Sp = psS.tile([128, 2, QC], F32, tag="S")
                            s_views = [Sp[:, 0, :], Sp[:, 1, :]]
                        else:
                            Sa = psS.tile([128, QC], F32, tag="Sa")
                            Sb = psS.tile([128, QC], F32, tag="Sb")
                            Sp = None
                            s_views = [Sa[:], Sb[:]]
                        mm_prev = None
                        for j, h in ((0, h0), (1, h1)):
                            pb = 64 * (h % 2)
                            coh = h // 2
                            mm = nc.tensor.matmul(
                                s_views[j],
                                kT[pb : pb + 64, coh, ch * QC + s * 128 : ch * QC + (s + 1) * 128],
                                qT[pb : pb + 64, coh, ch * QC : (ch + 1) * QC],
                                start=(j == 0 or not PAIR_S),
                                stop=True,
                                skip_group_check=(j == 1 and PAIR_S),
                            )
                            if PAIR_S and j == 1:
                                # the start=True matmul must clear the bank's
                                # has_written bits BEFORE the start=False one
                                # lands; they touch disjoint halves so no data
                                # dep exists -- order them explicitly.
                                add_dep_helper(
                                    mm.ins, mm_prev.ins, sync=True,
                                    reason="paired-S bank: start-clear first",
                                )
                            mm_prev = mm
                        PM = pm_pool.tile([128, 2, QC], F32R, tag="PM")
                        H = QC // 2
                        if ch == 0 or FULL_MASK:
                            # chunk 0: data-dependent masks (sequence start on
                            # first-half cores) -> full-width exp + mask
                            if PAIR_S:
                                nc.scalar.activation(PM[:], Sp[:], AF.Exp, scale=0.125)
                            else:
                                for j in (0, 1):
                                    nc.scalar.activation(
                                        PM[:, j, :], s_views[j], AF.Exp, scale=0.125
                                    )
                            nc.vector.tensor_tensor(
                                out=PM[:],
                                in0=PM[:],
                                in1=mask_sb[:, mset + s, :].unsqueeze(1).broadcast_to(
                                    [128, 2, QC]
                                ),
                                op=mybir.AluOpType.mult,
                            )
                        elif s in (0, 3):
                            # band mask kills one column half outright; exp
                            # only the live half, zero-fill the dead half
                            lo = 0 if s == 0 else H
                            dead = H if s == 0 else 0
                            for j in (0, 1):
                                nc.scalar.activation(
                                    PM[:, j, lo : lo + H],
                                    s_views[j][:, lo : lo + H],
                                    AF.Exp,
                                    scale=0.125,
                                )
                            nc.vector.tensor_tensor(
                                out=PM[:, :, lo : lo + H],
                                in0=PM[:, :, lo : lo + H],
                                in1=mask_sb[:, mset + s, lo : lo + H]
                                .unsqueeze(1)
                                .broadcast_to([128, 2, H]),
                                op=mybir.AluOpType.mult,
                            )
                            nc.vector.tensor_copy(
                                PM[:, :, dead : dead + H],
                                zeros_qc[:].rearrange("p (a b) -> p a b", a=2),
                            )
                        else:
                            # s=1: only cols [H:) touch the band edge;
                            # s=2: only cols [0:H). The other half is fully
                            # valid -> exp straight into PM, no mask there.
                            lo = H if s == 1 else 0
                            for j in (0, 1):
                                nc.scalar.activation(
                                    PM[:, j, :], s_views[j], AF.Exp, scale=0.125
                                )
                            nc.vector.tensor_tensor(
                                out=PM[:, :, lo : lo + H],
                                in0=PM[:, :, lo : lo + H],
                                in1=mask_sb[:, mset + s, lo : lo + H]
                                .unsqueeze(1)
                                .broadcast_to([128, 2, H]),
                                op=mybir.AluOpType.mult,
                            )
                        pms[s] = PM

                    for j, h in ((0, h0), (1, h1)):
                        yps = psY.tile([65, QC], F32, tag="y")
                        for s in range(4):
                            nc.tensor.matmul(
                                yps[:],
                                V[:, 2 * ch + s, (D + 1) * h : (D + 1) * (h + 1)],
                                pms[s][:, j, :],
                                start=(s == 0),
                                stop=(s == 3),
                            )
                        # rowsum row 64 -> SBUF (custom DVE ops can't read
                        # PSUM) and unnormalized y -> SBUF staging, freeing
                        # the PSUM tile immediately; then recip, broadcast,
                        # normalize out of SBUF.
                        pb = 64 * (h % 2)
                        rt = r_pool.tile([1, QC], F32, tag="rt")
                        nc.scalar.activation(rt[:], yps[64:65, :], AF.Copy)
                        yu = yu_pool.tile([128, QC], F32, tag="yu")
                        nc.vector.tensor_copy(yu[pb : pb + 64, :], yps[0:64, :])
                        Rr = r_pool.tile([1, QC], F32, tag="Rr")
                        nc.vector.reciprocal_approx_fast(Rr[:], rt[0:1, :])
                        rb = r_pool.tile([128, QC], F32, tag="rb")
                        nc.gpsimd.partition_broadcast(rb[:], Rr[0:1, :], channels=128)
                        nc.vector.tensor_tensor(
                            out=y_sb[pb : pb + 64, h // 2, :],
                            in0=yu[pb : pb + 64, :],
                            in1=rb[pb : pb + 64, :],
                            op=mybir.AluOpType.mult,
                        )

                for co in range(CO):
                    pps = psP.tile([128, QC], F32, tag="pj")
                    for ci in range(CO):
                        nc.tensor.matmul(
                            pps[:],
                            wp_sb[:, ci, co * 128 : (co + 1) * 128],
                            y_sb[:, ci, :],
                            start=(ci == 0),
                            stop=(ci == CO - 1),
                        )
                    o_sb = ostage_pool.tile([128, QC], F32, tag="ost")
                    nc.scalar.activation(o_sb[:], pps[:], AF.Copy)
                    nc.sync.dma_start(
                        outT[co * 128 : (co + 1) * 128, ch * QC : (ch + 1) * QC],
                        o_sb[:],
                    )


_BUILD_CACHE = {}


def build_bass(TQ=1024, enable_asserts=False):
    key = (TQ, enable_asserts)
    if key in _BUILD_CACHE:
        return _BUILD_CACHE[key]
    EXT = TQ + PAD
    nc = bacc.Bacc(
        "TRN2",
        target_bir_lowering=False,
        debug=False,
        enable_asserts=enable_asserts,
    )
    xT = nc.dram_tensor("xT", [C, EXT], F32R, kind="ExternalInput").ap()
    wqkvT = nc.dram_tensor("wqkvT", [C, 3 * C], F32R, kind="ExternalInput").ap()
    wprojT = nc.dram_tensor("wprojT", [C, C], F32R, kind="ExternalInput").ap()
    masks = nc.dram_tensor("masks", [2, 4, 128, QC], F32, kind="ExternalInput").ap()
    outT = nc.dram_tensor("outT", [C, TQ], F32, kind="ExternalOutput").ap()

    with tile.TileContext(nc) as tc:
        _build_body(tc, xT, wqkvT, wprojT, masks, outT, TQ)
    nc.compile()
    _BUILD_CACHE[key] = nc
    return nc


def make_masks(first_half: bool) -> np.ndarray:
    """0/1 band masks, [2 sets, 4 subtiles, 128 kr, QC qq].

    valid(kr, qq, s):  qq+1 <= 128*s+kr <= qq+256.
    Set 0 is used by chunk 0 (subtiles 0,1 zeroed on first-half cores:
    those keys fall before the sequence start); set 1 by chunks 1+.
    """
    kr = np.arange(128)[:, None]
    qq = np.arange(QC)[None, :]
    m = np.zeros((2, 4, 128, QC), dtype=np.float32)
    for s in range(4):
        kl = 128 * s + kr
        m[1, s] = ((qq + 1 <= kl) & (kl <= qq + 256)).astype(np.float32)
    m[0] = m[1]
    if first_half:
        m[0, 0] = 0.0
        m[0, 1] = 0.0
    return m


def _prep_core_inputs(x, wqkvT, wprojT, masks_by_half, b, half, TQ):
    start = half * TQ
    T = x.shape[1]
    ext = np.zeros((TQ + PAD, C), dtype=np.float32)
    lo = start - PAD
    src_lo = max(lo, 0)
    ext[src_lo - lo : TQ + PAD] = x[b, src_lo : start + TQ]
    return {
        "xT": np.ascontiguousarray(ext.T),
        "wqkvT": wqkvT,
        "wprojT": wprojT,
        "masks": masks_by_half[half],
    }


def kernel(x, W_qkv, W_proj):
    x = np.asarray(x, dtype=np.float32)
    W_qkv = np.asarray(W_qkv, dtype=np.float32)
    W_proj = np.asarray(W_proj, dtype=np.float32)
    B, T, _ = x.shape
    TQ = T // 2

    nc = build_bass(TQ)
    wqkvT = np.ascontiguousarray(W_qkv.T)
    wprojT = np.ascontiguousarray(W_proj.T)
    masks_by_half = {0: make_masks(True), 1: make_masks(False)}

    in_maps = [
        _prep_core_inputs(x, wqkvT, wprojT, masks_by_half, core // 2, core % 2, TQ)
        for core in range(8)
    ]
    res = bass_utils.run_bass_kernel_spmd(nc, in_maps, core_ids=list(range(8)))
    kernel.last_run_results = res

    out = np.empty((B, T, C), dtype=np.float32)
    for core in range(8):
        b, half = core // 2, core % 2
        out[b, half * TQ : (half + 1) * TQ, :] = res.results[core]["outT"].T
    return out



# revision 13
# speedup vs baseline: 1.3798x; 1.3798x over previous
"""Sliding-window causal self-attention for Trainium2, 8 NeuronCores.

Problem: B=4, T=2048, C=1024, 16 heads x 64 dim, window=256 causal band.
  qkv = x @ W_qkv.T ; windowed-causal attention ; out = y @ W_proj.T

Sharding: 8 cores = 4 batches x 2 sequence halves (1024 queries each),
zero cross-core communication; each core gets a 1280-row extended x^T
slice (256-row halo, zero-padded on first halves).

All matmuls bf16 (rel err ~3e-3 vs 2e-2 budget). Layout per core:
  - qkv projections (phase A) computed as [128, 256]-wide PSUM tiles
    with W resident in SBUF.  Only the tiles needed by attention
    chunk 0 run up front; the rest are interleaved one-per-unit into
    the attention stream as PE filler, so the tensor engine never
    waits on the softmax chain (exp -> mask -> AV dependency latency).
  - kT/qT head-dim-major bf16; V natural [t, c] bf16 augmented with a
    ones column per head (65 cols) so AV also yields softmax denoms.
  Attention, per (chunk of 256 queries, query-half, head) unit:
  - banded S^T scores: the 3 live key-subtiles [128k x 128q] into one
    PSUM bank [128, 3, 128] (slots j0, j2, j1: triangles contiguous).
  - exp on ScalarE (scale=1/8) -> PM bf16; 0/1 band masks multiplied
    on VectorE / GpSimd (triangle slots only, except chunk 0).
  - AV flipped: PM stationary, V_aug moving -> y [128 q, 65]
    accumulating 3 key-subtiles; 4 heads share one PSUM bank.
  - per 4-head group: stage y to SBUF, strided fast-recip of the
    denom column, one broadcast multiply (GpSimd) -> y_norm bf16;
    pair-transposes (2 heads at once, [128,128] bf16, 2 per PSUM
    tile) back to channel-major yT; the previous chunk's projection
    tiles are interleaved at group ends.
GpSimd (Pool) never touches PSUM (unsupported by codegen).
"""

import numpy as np
import ml_dtypes
from contextlib import ExitStack

import concourse.bass as bass
import concourse.tile as tile
import concourse.mybir as mybir
from concourse import bacc
from concourse.tile import add_dep_helper
from concourse import bass_utils

F32 = mybir.dt.float32
BF16 = mybir.dt.bfloat16
AF = mybir.ActivationFunctionType
MUL = mybir.AluOpType.mult
BF = ml_dtypes.bfloat16

C = 1024
HEADS = 16
D = 64
WINDOW = 256
PAD = WINDOW
CO = C // 128          # 8
TQ = 1024              # queries per core
EXT = TQ + PAD         # 1280
EO = EXT // 128        # 10
NCH = TQ // 256        # 4 chunks of 256 queries
SLOT_J = (0, 2, 1)     # S-tile slot -> key-subtile j (triangles first)


def _build_body(tc, xT, wqkvT, wprojT, masks, ident, outT):
    nc = tc.nc

    def chain(mm, prev):
        if prev is not None:
            add_dep_helper(mm.ins, prev.ins, sync=True, reason="psum bank order")
        return mm

    with ExitStack() as outer:
        const = outer.enter_context(tc.tile_pool(name="const", bufs=1))

        xt = const.tile([128, CO, EXT], BF16)
        wq = const.tile([128, CO, 3 * C], BF16)
        wp = const.tile([128, CO, C], BF16)
        kT = const.tile([128, CO, EXT], BF16)
        qT = const.tile([128, CO, TQ], BF16)
        V = const.tile([128, EO, HEADS, D + 1], BF16)
        mask_sb = const.tile([128, 4, 3, 128], BF16)
        ident_sb = const.tile([128, 128], BF16)
        ones_col = const.tile([128, 1], BF16)

        wqr = wqkvT.rearrange("(ki p) o -> p ki o", p=128)
        xtr = xT.rearrange("(ki p) t -> p ki t", p=128)

        # DMA order by first use; first rounds are small so the first
        # qkv tiles are gated by ~2 rounds of queue latency only.
        for ki in range(CO):
            nc.sync.dma_start(wq[:, ki, C : C + 512], wqr[:, ki, C : C + 512])
            nc.sync.dma_start(xt[:, ki, 0:256], xtr[:, ki, 0:256])
        for ki in range(CO):
            nc.sync.dma_start(wq[:, ki, C + 512 : 2 * C], wqr[:, ki, C + 512 : 2 * C])
            nc.sync.dma_start(xt[:, ki, 256:640], xtr[:, ki, 256:640])
        for ki in range(CO):
            nc.sync.dma_start(wq[:, ki, 0:C], wqr[:, ki, 0:C])
            nc.sync.dma_start(xt[:, ki, 640:EXT], xtr[:, ki, 640:EXT])
        for ki in range(CO):
            nc.sync.dma_start(wq[:, ki, 2 * C : 3 * C], wqr[:, ki, 2 * C : 3 * C])
        nc.sync.dma_start(wp[:], wprojT.rearrange("(ci p) o -> p ci o", p=128))
        nc.sync.dma_start(mask_sb[:], masks.rearrange("s q p j m -> p (s q) j m"))
        nc.sync.dma_start(ident_sb[:], ident)

        nc.gpsimd.memset(ones_col[:], 1.0)
        nc.gpsimd.tensor_copy(
            V[:, :, :, D], ones_col[:, 0:1].broadcast_to([128, EO, HEADS])
        )

        with ExitStack() as ctx:
            pm_pool = ctx.enter_context(tc.tile_pool(name="pm", bufs=3))
            ysb_pool = ctx.enter_context(tc.tile_pool(name="ysb", bufs=2))
            r_pool = ctx.enter_context(tc.tile_pool(name="rr", bufs=2))
            ynorm_pool = ctx.enter_context(tc.tile_pool(name="ynorm", bufs=4))
            yt_pool = ctx.enter_context(tc.tile_pool(name="yt", bufs=2))
            osb_pool = ctx.enter_context(tc.tile_pool(name="osb", bufs=3))
            psS = ctx.enter_context(tc.tile_pool(name="psS", bufs=3, space="PSUM"))
            psY = ctx.enter_context(tc.tile_pool(name="psY", bufs=2, space="PSUM"))
            psT = ctx.enter_context(tc.tile_pool(name="psT", bufs=1, space="PSUM"))
            psX = ctx.enter_context(tc.tile_pool(name="psX", bufs=2, space="PSUM"))

            # ---- phase A as 256-wide tiles, streamed through psX ----
            sidx = 0

            def emit_a_tile(kind, blk, co, off=0, w=256):
                nonlocal sidx
                ps = psX.tile([128, 256], F32, tag="x", name="psx")
                if kind == "k":
                    wcol, x0 = C + co * 128, blk * 256 + off
                    dst = kT[:, co, x0 : x0 + w]
                elif kind == "q":
                    wcol, x0 = co * 128, PAD + blk * 256 + off
                    dst = qT[:, co, blk * 256 + off : blk * 256 + off + w]
                if kind in ("k", "q"):
                    for ki in range(CO):
                        nc.tensor.matmul(
                            ps[:, 0:w],
                            wq[:, ki, wcol : wcol + 128],
                            xt[:, ki, x0 : x0 + w],
                            start=(ki == 0),
                            stop=(ki == CO - 1),
                        )
                    src = ps[:, 0:w]
                else:
                    eo, cb = blk, co
                    for ki in range(CO):
                        nc.tensor.matmul(
                            ps[:],
                            xt[:, ki, eo * 128 : (eo + 1) * 128],
                            wq[:, ki, 2 * C + cb * 256 : 2 * C + (cb + 1) * 256],
                            start=(ki == 0),
                            stop=(ki == CO - 1),
                        )
                    dst = V[:, eo, 4 * cb : 4 * cb + 4, 0:D]
                    src = ps[:].rearrange("p (h d) -> p h d", d=D)
                if sidx % 2 == 0:
                    nc.scalar.activation(dst, src, AF.Copy)
                else:
                    nc.vector.tensor_copy(dst, src)
                sidx += 1

            def k_tiles(kb, off=0, w=256):
                return [("k", kb, co, off, w) for co in range(CO)]

            def q_tiles(qb, off=0, w=256):
                return [("q", qb, co, off, w) for co in range(CO)]

            def v_tiles(eo):
                return [("v", eo, cb) for cb in range(4)]

            # chunk ch needs key blocks {ch, ch+1}, query block ch, V eo in
            # [2ch, 2ch+3].  The last key/query blocks are split so chunk 3
            # (which has no whole blocks left) still gets PE filler: the
            # halves feeding only (ch3, qh1) run during ch3-qh0.
            pre_b = k_tiles(0) + k_tiles(1) + q_tiles(0) + \
                v_tiles(0) + v_tiles(1) + v_tiles(2) + v_tiles(3)
            fills = {
                0: k_tiles(2) + q_tiles(1) + v_tiles(4) + v_tiles(5),
                1: k_tiles(3) + q_tiles(2) + v_tiles(6) + v_tiles(7),
                2: k_tiles(4, 0, 128) + q_tiles(3, 0, 128) + v_tiles(8),
                3: k_tiles(4, 128, 128) + q_tiles(3, 128, 128) + v_tiles(9),
            }

            for t in pre_b:
                emit_a_tile(*t)

            # ---- attention units ----
            units = [
                (ch, qh, h) for ch in range(NCH) for qh in range(2) for h in range(HEADS)
            ]
            S_of = {}
            PM_of = {}
            ynorm_of = {}
            yt_of = {}
            psY_cur = [None, None]

            def emit_scores(U):
                ch, qh, h = U
                S = psS.tile([128, 3, 128], F32, tag="S", name="S")
                pb, coh = 64 * (h % 2), h // 2
                q0 = ch * 256 + qh * 128
                prev = None
                for slot, j in enumerate(SLOT_J):
                    kl0 = ch * 256 + qh * 128 + j * 128
                    mm = nc.tensor.matmul(
                        S[:, slot, :],
                        kT[pb : pb + 64, coh, kl0 : kl0 + 128],
                        qT[pb : pb + 64, coh, q0 : q0 + 128],
                        start=(slot == 0),
                        stop=(slot == 2),
                        skip_group_check=(slot > 0),
                    )
                    prev = chain(mm, prev)
                S_of[U] = S

            def emit_expmask(U, i):
                ch, qh, h = U
                PM = pm_pool.tile([128, 3, 128], BF16, tag="PM", name="PM")
                nc.scalar.activation(PM[:], S_of[U][:], AF.Exp, scale=0.125)
                del S_of[U]
                mrow = (0 if ch == 0 else 1) * 2 + qh
                if ch == 0:
                    nc.vector.tensor_tensor(
                        out=PM[:], in0=PM[:], in1=mask_sb[:, mrow, :, :], op=MUL
                    )
                else:
                    eng = nc.vector if i % 2 == 0 else nc.gpsimd
                    eng.tensor_tensor(
                        out=PM[:, 0:2, :],
                        in0=PM[:, 0:2, :],
                        in1=mask_sb[:, mrow, 0:2, :],
                        op=MUL,
                    )
                PM_of[U] = PM

            def emit_av(U):
                ch, qh, h = U
                g = h % 4
                if g == 0:
                    psY_cur[0] = psY.tile([128, 4, D + 1], F32, tag="Y", name="Yt")
                    psY_cur[1] = None
                yt, prev = psY_cur
                for slot, j in enumerate(SLOT_J):
                    eo = 2 * ch + qh + j
                    first = g == 0 and slot == 0
                    mm = nc.tensor.matmul(
                        yt[:, g, :],
                        PM_of[U][:, slot, :],
                        V[:, eo, h, :],
                        start=first,
                        stop=(g == 3 and slot == 2),
                        skip_group_check=(not first),
                    )
                    prev = chain(mm, prev)
                psY_cur[1] = prev
                del PM_of[U]

            def emit_proj_half(ch, qh, co):
                # one [128, 128] output tile: co-th channel block of the
                # (ch, qh) query half (shares the psX bank rotation, so
                # allocated at [128, 256] and half-used)
                psf = psX.tile([128, 256], F32, tag="x", name="psx")
                yt = yt_of[ch]
                for ci in range(CO):
                    nc.tensor.matmul(
                        psf[:, 0:128],
                        wp[:, ci, co * 128 : (co + 1) * 128],
                        yt[:, ci, qh * 128 : qh * 128 + 128],
                        start=(ci == 0),
                        stop=(ci == CO - 1),
                    )
                o_sb = osb_pool.tile([128, 128], F32, tag="o", name="o_sb")
                if co % 2 == 0:
                    nc.scalar.activation(o_sb[:], psf[:, 0:128], AF.Copy)
                else:
                    nc.vector.tensor_copy(o_sb[:], psf[:, 0:128])
                q0 = ch * 256 + qh * 128
                nc.sync.dma_start(
                    outT[co * 128 : (co + 1) * 128, q0 : q0 + 128], o_sb[:]
                )

            proj_queue = []      # ready (ch, qh, co) proj-half work
            pending_transp = []  # (ch, qh, hq0) transposes delayed 1 group

            def emit_transp(ch, qh, hq0):
                ynq = ynorm_of[(ch, qh)]
                if ch not in yt_of:
                    yt_of[ch] = yt_pool.tile([128, CO, 256], BF16, tag="yT", name="yTt")
                yTt = yt_of[ch]
                # two pair-transposes (heads hq0..hq0+3) share one psT bank
                pst = psT.tile([128, 2, 128], BF16, tag="T", name="pst")
                prev = None
                for u, hp in enumerate((hq0 // 2, hq0 // 2 + 1)):
                    mm = nc.tensor.matmul(
                        pst[:, u, :],
                        ynq[:, 2 * hp : 2 * hp + 2, :].rearrange("p a b -> p (a b)"),
                        ident_sb[:],
                        is_transpose=True,
                        start=(u == 0),
                        stop=(u == 1),
                        skip_group_check=(u == 1),
                    )
                    prev = chain(mm, prev)
                nc.vector.tensor_copy(
                    yTt[:, hq0 // 2 : hq0 // 2 + 2, qh * 128 : qh * 128 + 128],
                    pst[:].rearrange("p u m -> p u m"),
                )
                if hq0 == HEADS - 4:
                    # (ch, qh) fully transposed: its projection becomes ready
                    proj_queue.extend((ch, qh, co) for co in range(CO))

            def flush_transp():
                while pending_transp:
                    emit_transp(*pending_transp.pop(0))

            def emit_groupend(U, fill_thunk):
                ch, qh, h = U
                hq0 = h - 3
                y_sb = ysb_pool.tile([128, 4, D + 1], F32, tag="ysb", name="y_sb")
                nc.vector.tensor_copy(y_sb[:], psY_cur[0][:])
                r_sb = r_pool.tile([128, 4], F32, tag="r", name="r_sb")
                nc.vector.reciprocal_approx_fast(r_sb[:], y_sb[:, :, D])
                if (ch, qh) not in ynorm_of:
                    ynorm_of[(ch, qh)] = ynorm_pool.tile(
                        [128, HEADS, D], BF16, tag="yn", name="ynq"
                    )
                ynq = ynorm_of[(ch, qh)]
                nc.gpsimd.tensor_tensor(
                    out=ynq[:, hq0 : hq0 + 4, :],
                    in0=y_sb[:, :, 0:D],
                    in1=r_sb[:].unsqueeze(2).broadcast_to([128, 4, D]),
                    op=MUL,
                )
                # ready projection half + filler occupy PE while the
                # softmax-norm chain completes; the delayed transposes of
                # the PREVIOUS group then see a fully-written y_norm
                if proj_queue:
                    emit_proj_half(*proj_queue.pop(0))
                if fill_thunk is not None:
                    fill_thunk()
                flush_transp()
                pending_transp.append((ch, qh, hq0))

            emit_scores(units[0])
            emit_scores(units[1])
            fill_done = 0
            for i, U in enumerate(units):
                ch = U[0]
                emit_expmask(U, i)
                if i + 2 < len(units):
                    emit_scores(units[i + 2])
                # schedule phase-A filler tiles; chunk 3's fills feed only
                # (ch3, qh1) and must finish early in qh0
                iu = i % 32
                fl = fills[ch]
                span = 12 if ch == NCH - 1 else 32
                want = min(len(fl), ((min(iu, span - 1) + 1) * len(fl) + span - 1) // span)
                fill_thunk = None
                if fill_done < want:
                    t = fl[fill_done]
                    fill_thunk = lambda t=t: emit_a_tile(*t)
                    fill_done += 1
                is_ge = U[2] % 4 == 3
                if U[2] % 4 == 1 and proj_queue:
                    emit_proj_half(*proj_queue.pop(0))
                if not is_ge and fill_thunk is not None:
                    fill_thunk()
                    fill_thunk = None
                while fill_done < want:
                    emit_a_tile(*fl[fill_done])
                    fill_done += 1
                if iu == 31:
                    fill_done = 0
                emit_av(U)
                if is_ge:
                    emit_groupend(U, fill_thunk)
            flush_transp()
            while proj_queue:
                emit_proj_half(*proj_queue.pop(0))


_BUILD_CACHE = {}


def build_bass(TQ_arg=1024, enable_asserts=False):
    key = (TQ_arg, enable_asserts)
    if key in _BUILD_CACHE:
        return _BUILD_CACHE[key]
    assert TQ_arg == TQ
    nc = bacc.Bacc(
        "TRN2",
        target_bir_lowering=False,
        debug=False,
        enable_asserts=enable_asserts,
    )
    xT = nc.dram_tensor("xT", [C, EXT], BF16, kind="ExternalInput").ap()
    wqkvT = nc.dram_tensor("wqkvT", [C, 3 * C], BF16, kind="ExternalInput").ap()
    wprojT = nc.dram_tensor("wprojT", [C, C], BF16, kind="ExternalInput").ap()
    masks = nc.dram_tensor("masks", [2, 2, 128, 3, 128], BF16, kind="ExternalInput").ap()
    ident = nc.dram_tensor("ident", [128, 128], BF16, kind="ExternalInput").ap()
    outT = nc.dram_tensor("outT", [C, TQ], F32, kind="ExternalOutput").ap()

    with tile.TileContext(nc) as tc:
        _build_body(tc, xT, wqkvT, wprojT, masks, ident, outT)
    nc.compile()
    _BUILD_CACHE[key] = nc
    return nc


def make_masks(first_half: bool) -> np.ndarray:
    """0/1 band masks, [2 sets, 2 qh, 128 kr, 3 slots, 128 qq] bf16.

    Slot order (0, 2, 1): slot0 = key-subtile j0 (upper triangle
    kr >= qq+1), slot1 = j2 (lower triangle kr <= qq), slot2 = j1
    (all-valid; only multiplied for chunk 0).  Set 0 = chunk 0 (first
    halves zero the halo subtiles), set 1 = chunks 1+.
    """
    kr = np.arange(128)[:, None]
    qq = np.arange(128)[None, :]
    tri_up = (kr >= qq + 1).astype(np.float32)
    tri_lo = (kr <= qq).astype(np.float32)
    ones = np.ones((128, 128), np.float32)
    zeros = np.zeros((128, 128), np.float32)
    m = np.zeros((2, 2, 128, 3, 128), np.float32)
    for qh in range(2):
        m[1, qh, :, 0], m[1, qh, :, 1], m[1, qh, :, 2] = tri_up, tri_lo, ones
    if first_half:
        # chunk 0: j0 always halo; j1 halo for qh0
        m[0, 0, :, 0], m[0, 0, :, 1], m[0, 0, :, 2] = zeros, tri_lo, zeros
        m[0, 1, :, 0], m[0, 1, :, 1], m[0, 1, :, 2] = zeros, tri_lo, ones
    else:
        m[0] = m[1]
    return m.astype(BF)


def _prep_core_inputs(x, wqkvT, wprojT, masks_by_half, ident, b, half):
    start = half * TQ
    ext = np.zeros((EXT, C), dtype=np.float32)
    lo = start - PAD
    src_lo = max(lo, 0)
    ext[src_lo - lo : EXT] = x[b, src_lo : start + TQ]
    return {
        "xT": np.ascontiguousarray(ext.T).astype(BF),
        "wqkvT": wqkvT,
        "wprojT": wprojT,
        "masks": masks_by_half[half],
        "ident": ident,
    }


def kernel(x, W_qkv, W_proj):
    x = np.asarray(x, dtype=np.float32)
    W_qkv = np.asarray(W_qkv, dtype=np.float32)
    W_proj = np.asarray(W_proj, dtype=np.float32)
    B, T, _ = x.shape
    assert T == 2 * TQ

    nc = build_bass(TQ)
    wqkvT = np.ascontiguousarray(W_qkv.T).astype(BF)
    wprojT = np.ascontiguousarray(W_proj.T).astype(BF)
    masks_by_half = {0: make_masks(True), 1: make_masks(False)}
    ident = np.eye(128, dtype=BF)

    in_maps = [
        _prep_core_inputs(x, wqkvT, wprojT, masks_by_half, ident, core // 2, core % 2)
        for core in range(8)
    ]
    res = bass_utils.run_bass_kernel_spmd(nc, in_maps, core_ids=list(range(8)))
    kernel.last_run_results = res

    out = np.empty((B, T, C), dtype=np.float32)
    for core in range(8):
        b, half = core // 2, core % 2
        out[b, half * TQ : (half + 1) * TQ, :] = res.results[core]["outT"].T
    return out
